# revision 10
# baseline (speedup 1.0000x reference)
"""ClusterInversionLoss Trainium2 kernel.

Strategy (data-parallel over the flat pair list, per sharding hint):
  - Host: co-locate each pair's data by gathering rows at pair_i/pair_j
    (logits per class, target delta, sample weights), shard the 2M pairs
    evenly across 8 cores, lay out per-core planes as (128, 13, L) bf16.
  - Device (per core, pure streaming, no random access):
      s = softmax-expected-score of the 5 logits for both pair sides
      (exp on ACT; class-weighted sums via scalar_tensor_tensor FMAs;
      division via exp(-ln Z) to stay inside one ACT table set),
      softplus ranking loss via exp/ln with the free affine bias,
      fused multiply+reduce into per-partition accumulators.
  - Host: sum the 8x128 partial (loss, weight) pairs, return the ratio.

Computes exactly the reference quantity; only rows referenced by pairs
contribute to the loss, so unpaired rows need not be touched.
"""

import math

import numpy as np

import concourse.bacc as bacc
import concourse.mybir as mybir
from concourse.bass_utils import run_bass_kernel_spmd
from concourse.tile import TileContext

NCORES = 8
NPAIRS = 2_000_000
PC = NPAIRS // NCORES  # 250_000 pairs per core
P = 128
NCHUNK = 2
LC = 978
L = NCHUNK * LC  # 1956 columns per partition; P*L = 250_368 >= PC
NPLANE = 13  # li0..li4, lj0..lj4, dy, wi, wj

EPS = 1e-8

f32 = mybir.dt.float32
bf16 = mybir.dt.bfloat16
AF = mybir.ActivationFunctionType
ALU = mybir.AluOpType


import os

FLAGS = {
    "pool_adds": os.environ.get("K_POOL_ADDS", "1") == "1",
    "use_stt": os.environ.get("K_USE_STT", "1") == "1",
}


def _build():
    nc = bacc.Bacc("TRN2", target_bir_lowering=False)
    X = nc.dram_tensor("x", [P, NPLANE, L], bf16, kind="ExternalInput")
    OUT = nc.dram_tensor("out", [P, 2], f32, kind="ExternalOutput")

    with TileContext(nc) as tc:
        with (
            tc.tile_pool(name="io", bufs=2) as io,
            tc.tile_pool(name="ew", bufs=2) as ew,
            tc.tile_pool(name="sc", bufs=1) as sc,
            tc.tile_pool(name="acc", bufs=1) as accp,
        ):
            accL = [accp.tile([P, 1], f32, tag=f"accL{c}", name=f"accL{c}")
                    for c in range(NCHUNK)]
            accW = [accp.tile([P, 1], f32, tag=f"accW{c}", name=f"accW{c}")
                    for c in range(NCHUNK)]

            for c in range(NCHUNK):
                cs = slice(c * LC, (c + 1) * LC)

                LI = io.tile([P, 5, LC], bf16, tag="LI")
                nc.sync.dma_start(out=LI[:], in_=X[:, 0:5, cs])
                LJ = io.tile([P, 5, LC], bf16, tag="LJ")
                nc.sync.dma_start(out=LJ[:], in_=X[:, 5:10, cs])
                DY = io.tile([P, LC], bf16, tag="DY")
                nc.sync.dma_start(out=DY[:], in_=X[:, 10, cs])
                WI = io.tile([P, LC], bf16, tag="WI")
                nc.sync.dma_start(out=WI[:], in_=X[:, 11, cs])
                WJ = io.tile([P, LC], bf16, tag="WJ")
                nc.sync.dma_start(out=WJ[:], in_=X[:, 12, cs])

                # e = exp(logits), one big ACT op per side
                EI = ew.tile([P, 5, LC], f32, tag="EI")
                nc.scalar.activation(EI[:], LI[:], AF.Exp)
                EJ = ew.tile([P, 5, LC], f32, tag="EJ")
                nc.scalar.activation(EJ[:], LJ[:], AF.Exp)

                # Z = sum_c e_c
                zeng = nc.gpsimd if FLAGS["pool_adds"] else nc.vector
                Zi = sc.tile([P, LC], f32, tag="Zi")
                zeng.tensor_add(out=Zi[:], in0=EI[:, 0, :], in1=EI[:, 1, :])
                zeng.tensor_add(out=Zi[:], in0=Zi[:], in1=EI[:, 2, :])
                zeng.tensor_add(out=Zi[:], in0=Zi[:], in1=EI[:, 3, :])
                zeng.tensor_add(out=Zi[:], in0=Zi[:], in1=EI[:, 4, :])
                Zj = sc.tile([P, LC], f32, tag="Zj")
                zeng.tensor_add(out=Zj[:], in0=EJ[:, 0, :], in1=EJ[:, 1, :])
                zeng.tensor_add(out=Zj[:], in0=Zj[:], in1=EJ[:, 2, :])
                zeng.tensor_add(out=Zj[:], in0=Zj[:], in1=EJ[:, 3, :])
                zeng.tensor_add(out=Zj[:], in0=Zj[:], in1=EJ[:, 4, :])

                # W = e1 + 2*e2 + 3*e3 + 4*e4
                Wi = sc.tile([P, LC], f32, tag="Wi")
                Wj = sc.tile([P, LC], f32, tag="Wj")
                if FLAGS["use_stt"]:
                    for W, E in ((Wi, EI), (Wj, EJ)):
                        nc.vector.scalar_tensor_tensor(
                            out=W[:], in0=E[:, 2, :], scalar=2.0, in1=E[:, 1, :],
                            op0=ALU.mult, op1=ALU.add)
                        nc.vector.scalar_tensor_tensor(
                            out=W[:], in0=E[:, 3, :], scalar=3.0, in1=W[:],
                            op0=ALU.mult, op1=ALU.add)
                        nc.vector.scalar_tensor_tensor(
                            out=W[:], in0=E[:, 4, :], scalar=4.0, in1=W[:],
                            op0=ALU.mult, op1=ALU.add)
                else:
                    T = sc.tile([P, LC], f32, tag="Tw")
                    for W, E in ((Wi, EI), (Wj, EJ)):
                        nc.vector.tensor_scalar_mul(out=W[:], in0=E[:, 2, :],
                                                    scalar1=2.0)
                        nc.vector.tensor_add(out=W[:], in0=W[:], in1=E[:, 1, :])
                        nc.vector.tensor_scalar_mul(out=T[:], in0=E[:, 3, :],
                                                    scalar1=3.0)
                        nc.vector.tensor_add(out=W[:], in0=W[:], in1=T[:])
                        nc.vector.tensor_scalar_mul(out=T[:], in0=E[:, 4, :],
                                                    scalar1=4.0)
                        nc.vector.tensor_add(out=W[:], in0=W[:], in1=T[:])

                # 1/Z = exp(-ln Z) on ACT (stays in the
                # natural_log_exp_and_others table set).
                # ACT must not write its own input tile (HW exec-unit
                # crash), so ping-pong through a scratch tile.
                ZT = sc.tile([P, LC], f32, tag="ZT")
                nc.scalar.activation(ZT[:], Zi[:], AF.Ln)
                nc.scalar.activation(Zi[:], ZT[:], AF.Exp, scale=-1.0)
                nc.scalar.activation(ZT[:], Zj[:], AF.Ln)
                nc.scalar.activation(Zj[:], ZT[:], AF.Exp, scale=-1.0)

                # s_i, s_j, ds = s_i - s_j
                nc.vector.tensor_mul(out=Wi[:], in0=Wi[:], in1=Zi[:])
                nc.vector.tensor_mul(out=Wj[:], in0=Wj[:], in1=Zj[:])
                nc.vector.tensor_sub(out=Wi[:], in0=Wi[:], in1=Wj[:])

                S1 = sc.tile([P, LC], f32, tag="S1")
                S2 = sc.tile([P, LC], f32, tag="S2")
                S3 = sc.tile([P, LC], f32, tag="S3")
                S4 = sc.tile([P, LC], f32, tag="S4")
                S5 = sc.tile([P, LC], f32, tag="S5")

                # sg = (dy > 0) - (dy < 0)
                nc.vector.tensor_scalar(
                    out=S1[:], in0=DY[:], scalar1=0.0, scalar2=None, op0=ALU.is_lt)
                if FLAGS["use_stt"]:
                    nc.vector.scalar_tensor_tensor(
                        out=S2[:], in0=DY[:], scalar=0.0, in1=S1[:],
                        op0=ALU.is_gt, op1=ALU.subtract)
                else:
                    nc.vector.tensor_scalar(
                        out=S2[:], in0=DY[:], scalar1=0.0, scalar2=None,
                        op0=ALU.is_gt)
                    nc.vector.tensor_sub(out=S2[:], in0=S2[:], in1=S1[:])
                # delta = sg * ds
                nc.vector.tensor_mul(out=S1[:], in0=S2[:], in1=Wi[:])
                # softplus(-delta) = ln(1 + exp(-delta))
                nc.scalar.activation(S2[:], S1[:], AF.Exp, scale=-1.0)
                nc.scalar.activation(S1[:], S2[:], AF.Ln, bias=1.0)
                # dist = |dy| = max(-dy, dy)
                if FLAGS["use_stt"]:
                    nc.vector.scalar_tensor_tensor(
                        out=S2[:], in0=DY[:], scalar=-1.0, in1=DY[:],
                        op0=ALU.mult, op1=ALU.max)
                else:
                    nc.vector.tensor_scalar_mul(out=S2[:], in0=DY[:], scalar1=-1.0)
                    nc.vector.tensor_max(out=S2[:], in0=S2[:], in1=DY[:])
                # active = min(dist, 1)
                nc.vector.tensor_scalar(
                    out=S3[:], in0=S2[:], scalar1=1.0, scalar2=None, op0=ALU.min)
                # wsum = wi + wj ; coeff = dist * wsum
                nc.vector.tensor_add(out=S4[:], in0=WI[:], in1=WJ[:])
                nc.vector.tensor_mul(out=S2[:], in0=S2[:], in1=S4[:])

                # total_loss_c = sum(0.5 * softplus * coeff) via
                # scalar_tensor_tensor's fused accum_out
                nc.vector.scalar_tensor_tensor(
                    out=S5[:], in0=S1[:], scalar=0.5, in1=S2[:],
                    op0=ALU.mult, op1=ALU.mult, accum_out=accL[c][:])
                # total_weight_c = sum(0.5 * active * wsum)
                nc.vector.scalar_tensor_tensor(
                    out=S1[:], in0=S3[:], scalar=0.5, in1=S4[:],
                    op0=ALU.mult, op1=ALU.mult, accum_out=accW[c][:])
                if c > 0:
                    nc.vector.tensor_add(out=accL[c][:], in0=accL[c][:],
                                         in1=accL[c - 1][:])
                    nc.vector.tensor_add(out=accW[c][:], in0=accW[c][:],
                                         in1=accW[c - 1][:])

            nc.sync.dma_start(out=OUT[:, 0:1], in_=accL[NCHUNK - 1][:])
            nc.sync.dma_start(out=OUT[:, 1:2], in_=accW[NCHUNK - 1][:])

    nc.compile()
    return nc


_NC_CACHE = {}


def _get_nc():
    if "nc" not in _NC_CACHE:
        _NC_CACHE["nc"] = _build()
    return _NC_CACHE["nc"]


def _prepare(inputs, targets, cluster_ids, sample_weight, pair_i, pair_j):
    import ml_dtypes

    x = np.ascontiguousarray(np.asarray(inputs), dtype=np.float32)
    t = np.asarray(targets)
    w = np.asarray(sample_weight, dtype=np.float32)
    pi = np.asarray(pair_i).astype(np.int64, copy=False)
    pj = np.asarray(pair_j).astype(np.int64, copy=False)

    li = x[pi]  # (NPAIRS, 5)
    lj = x[pj]
    dy = (t[pi] - t[pj]).astype(np.float32)
    wi = w[pi]
    wj = w[pj]

    PL = P * L
    bf = ml_dtypes.bfloat16
    maps = []
    for k in range(NCORES):
        sl = slice(k * PC, (k + 1) * PC)
        A = np.zeros((P, NPLANE, L), dtype=bf)

        def put(plane, v):
            vv = np.zeros(PL, dtype=np.float32)
            vv[:PC] = v
            A[:, plane, :] = vv.reshape(P, L).astype(bf)

        lis = li[sl]
        ljs = lj[sl]
        for ccls in range(5):
            put(ccls, lis[:, ccls])
            put(5 + ccls, ljs[:, ccls])
        put(10, dy[sl])
        put(11, wi[sl])
        put(12, wj[sl])
        maps.append({"x": A})
    return maps


def _run(in_maps, trace=False, **kw):
    nc = _get_nc()
    return run_bass_kernel_spmd(nc, in_maps, list(range(NCORES)), trace=trace, **kw)


def kernel(inputs, targets, cluster_ids, sample_weight, pair_i, pair_j):
    in_maps = _prepare(inputs, targets, cluster_ids, sample_weight, pair_i, pair_j)
    res = _run(in_maps)
    tl = 0.0
    tw = 0.0
    for k in range(NCORES):
        o = res.results[k]["out"]
        tl += float(o[:, 0].sum(dtype=np.float64))
        tw += float(o[:, 1].sum(dtype=np.float64))
    return np.float32(tl / (tw + EPS))


# revision 11
# speedup vs baseline: 1.1629x; 1.1629x over previous
"""ClusterInversionLoss Trainium2 kernel.

Strategy (data-parallel over the flat pair list, per sharding hint):
  - Host: co-locate each pair's data by gathering rows at pair_i/pair_j
    (logits per class, target delta, sample weights), shard the 2M pairs
    evenly across 8 cores, lay out per-core planes as (128, 13, L) bf16.
  - Device (per core, pure streaming, no random access):
      s = softmax-expected-score of the 5 logits for both pair sides
      (exp on ACT; class-weighted sums via scalar_tensor_tensor FMAs;
      division via exp(-ln Z) to stay inside one ACT table set),
      softplus ranking loss via exp/ln with the free affine bias,
      fused multiply+reduce into per-partition accumulators.
  - Host: sum the 8x128 partial (loss, weight) pairs, return the ratio.

Computes exactly the reference quantity; only rows referenced by pairs
contribute to the loss, so unpaired rows need not be touched.
"""

import os

import numpy as np

import concourse.bacc as bacc
import concourse.mybir as mybir
from concourse.bass_utils import run_bass_kernel_spmd
from concourse.tile import TileContext

NCORES = 8
NPAIRS = 2_000_000
PC = NPAIRS // NCORES  # 250_000 pairs per core
P = 128
NCHUNK = 2
LC = 978
L = NCHUNK * LC  # 1956 columns per partition; P*L = 250_368 >= PC
NPLANE = 13  # li0..li4, lj0..lj4, dy, wi, wj

EPS = 1e-8

f32 = mybir.dt.float32
bf16 = mybir.dt.bfloat16
AF = mybir.ActivationFunctionType
ALU = mybir.AluOpType


def _pin_act_tables(arch):
    """Make every ACT function we use first-match to one table set that
    contains both exp and ln, so the kernel needs a single
    ACT_TABLE_LOAD instead of thrashing between the exp-only and
    ln-only sets (1.3us per reload).  Only membership of the cached
    selection dict is edited; set indices (act_func_set_id) and the
    real on-device tables are untouched, so lowering stays correct.
    """
    from concourse.hw_specs import get_activation_tables

    tabs = get_activation_tables(arch)
    ours = {AF.Exp, AF.Ln, AF.Sign, AF.Abs, AF.Square}
    combined = None
    for name, fns in tabs.items():
        if ours <= fns:
            combined = name
            break
    if combined is None:
        return
    for name, fns in tabs.items():
        if name != combined:
            fns -= ours


def _build():
    nc = bacc.Bacc("TRN2", target_bir_lowering=False)
    _pin_act_tables(nc.m.arch)
    X = nc.dram_tensor("x", [P, NPLANE, L], bf16, kind="ExternalInput")
    OUT = nc.dram_tensor("out", [P, 2], f32, kind="ExternalOutput")

    with TileContext(nc) as tc:
        with (
            tc.tile_pool(name="io", bufs=2) as io,
            tc.tile_pool(name="ew", bufs=2) as ew,
            tc.tile_pool(name="sc", bufs=2) as sc,
            tc.tile_pool(name="acc", bufs=1) as accp,
        ):
            accL = [accp.tile([P, 1], f32, tag=f"accL{c}", name=f"accL{c}")
                    for c in range(NCHUNK)]
            accW = [accp.tile([P, 1], f32, tag=f"accW{c}", name=f"accW{c}")
                    for c in range(NCHUNK)]

            for c in range(NCHUNK):
                cs = slice(c * LC, (c + 1) * LC)

                LI = io.tile([P, 5, LC], bf16, tag="LI")
                nc.sync.dma_start(out=LI[:], in_=X[:, 0:5, cs])
                LJ = io.tile([P, 5, LC], bf16, tag="LJ")
                nc.sync.dma_start(out=LJ[:], in_=X[:, 5:10, cs])
                DY = io.tile([P, LC], bf16, tag="DY")
                nc.sync.dma_start(out=DY[:], in_=X[:, 10, cs])
                WI = io.tile([P, LC], bf16, tag="WI")
                nc.sync.dma_start(out=WI[:], in_=X[:, 11, cs])
                WJ = io.tile([P, LC], bf16, tag="WJ")
                nc.sync.dma_start(out=WJ[:], in_=X[:, 12, cs])

                # e = exp(logits), one big ACT op per side, bf16 out
                EI = ew.tile([P, 5, LC], bf16, tag="EI")
                nc.scalar.activation(EI[:], LI[:], AF.Exp)
                EJ = ew.tile([P, 5, LC], bf16, tag="EJ")
                nc.scalar.activation(EJ[:], LJ[:], AF.Exp)

                # Z = sum_c e_c  (bf16 adds; side i on Pool, side j on DVE)
                Zi = sc.tile([P, LC], bf16, tag="Zi")
                nc.gpsimd.tensor_add(out=Zi[:], in0=EI[:, 0, :], in1=EI[:, 1, :])
                nc.gpsimd.tensor_add(out=Zi[:], in0=Zi[:], in1=EI[:, 2, :])
                nc.gpsimd.tensor_add(out=Zi[:], in0=Zi[:], in1=EI[:, 3, :])
                nc.gpsimd.tensor_add(out=Zi[:], in0=Zi[:], in1=EI[:, 4, :])
                Zj = sc.tile([P, LC], bf16, tag="Zj")
                nc.vector.tensor_add(out=Zj[:], in0=EJ[:, 0, :], in1=EJ[:, 1, :])
                nc.vector.tensor_add(out=Zj[:], in0=Zj[:], in1=EJ[:, 2, :])
                nc.vector.tensor_add(out=Zj[:], in0=Zj[:], in1=EJ[:, 3, :])
                nc.vector.tensor_add(out=Zj[:], in0=Zj[:], in1=EJ[:, 4, :])

                # W = e1 + 2*e2 + 3*e3 + 4*e4 via FMA chain (bf16)
                Wi = sc.tile([P, LC], bf16, tag="Wi")
                Wj = sc.tile([P, LC], bf16, tag="Wj")
                for W, E in ((Wi, EI), (Wj, EJ)):
                    nc.vector.scalar_tensor_tensor(
                        out=W[:], in0=E[:, 2, :], scalar=2.0, in1=E[:, 1, :],
                        op0=ALU.mult, op1=ALU.add)
                    nc.vector.scalar_tensor_tensor(
                        out=W[:], in0=E[:, 3, :], scalar=3.0, in1=W[:],
                        op0=ALU.mult, op1=ALU.add)
                    nc.vector.scalar_tensor_tensor(
                        out=W[:], in0=E[:, 4, :], scalar=4.0, in1=W[:],
                        op0=ALU.mult, op1=ALU.add)

                # 1/Z = exp(-ln Z) on ACT; sign/abs of dy on ACT too
                ZT = sc.tile([P, LC], f32, tag="ZT")
                RZi = sc.tile([P, LC], f32, tag="RZi")
                RZj = sc.tile([P, LC], f32, tag="RZj")
                nc.scalar.activation(ZT[:], Zi[:], AF.Ln)
                nc.scalar.activation(RZi[:], ZT[:], AF.Exp, scale=-1.0)
                nc.scalar.activation(ZT[:], Zj[:], AF.Ln)
                nc.scalar.activation(RZj[:], ZT[:], AF.Exp, scale=-1.0)
                SG = sc.tile([P, LC], f32, tag="SG")
                nc.scalar.activation(SG[:], DY[:], AF.Sign)
                DIST = sc.tile([P, LC], f32, tag="DIST")
                nc.scalar.activation(DIST[:], DY[:], AF.Abs)

                # s_i, s_j, ds = s_i - s_j  (DVE)
                Si = sc.tile([P, LC], f32, tag="Si")
                Sj = sc.tile([P, LC], f32, tag="Sj")
                nc.vector.tensor_mul(out=Si[:], in0=Wi[:], in1=RZi[:])
                nc.vector.tensor_mul(out=Sj[:], in0=Wj[:], in1=RZj[:])
                nc.vector.tensor_sub(out=Si[:], in0=Si[:], in1=Sj[:])

                S1 = sc.tile([P, LC], f32, tag="S1")
                S2 = sc.tile([P, LC], f32, tag="S2")
                S3 = sc.tile([P, LC], f32, tag="S3")
                WS = sc.tile([P, LC], bf16, tag="WS")

                # delta = sg * ds (Pool); wsum/coeff on Pool as well
                nc.gpsimd.tensor_mul(out=S1[:], in0=SG[:], in1=Si[:])
                # softplus(-delta) = ln(1 + exp(-delta))
                nc.scalar.activation(S2[:], S1[:], AF.Exp, scale=-1.0)
                nc.scalar.activation(S1[:], S2[:], AF.Ln, bias=1.0)
                # active = min(dist, 1)
                nc.vector.tensor_scalar(
                    out=S3[:], in0=DIST[:], scalar1=1.0, scalar2=None, op0=ALU.min)
                # wsum = wi + wj ; coeff = dist * wsum
                nc.gpsimd.tensor_add(out=WS[:], in0=WI[:], in1=WJ[:])
                nc.gpsimd.tensor_mul(out=S2[:], in0=DIST[:], in1=WS[:])

                # total_loss_c = sum(softplus * coeff) (0.5 folded on host)
                S5 = sc.tile([P, LC], f32, tag="S5")
                nc.vector.scalar_tensor_tensor(
                    out=S5[:], in0=S1[:], scalar=1.0, in1=S2[:],
                    op0=ALU.mult, op1=ALU.mult, accum_out=accL[c][:])
                # total_weight_c = sum(active * wsum)
                nc.vector.scalar_tensor_tensor(
                    out=S1[:], in0=S3[:], scalar=1.0, in1=WS[:],
                    op0=ALU.mult, op1=ALU.mult, accum_out=accW[c][:])
                if c > 0:
                    nc.vector.tensor_add(out=accL[c][:], in0=accL[c][:],
                                         in1=accL[c - 1][:])
                    nc.vector.tensor_add(out=accW[c][:], in0=accW[c][:],
                                         in1=accW[c - 1][:])

            nc.sync.dma_start(out=OUT[:, 0:1], in_=accL[NCHUNK - 1][:])
            nc.sync.dma_start(out=OUT[:, 1:2], in_=accW[NCHUNK - 1][:])

    nc.compile()
    return nc


_NC_CACHE = {}


def _get_nc():
    if "nc" not in _NC_CACHE:
        _NC_CACHE["nc"] = _build()
    return _NC_CACHE["nc"]


def _prepare(inputs, targets, cluster_ids, sample_weight, pair_i, pair_j):
    import ml_dtypes

    x = np.ascontiguousarray(np.asarray(inputs), dtype=np.float32)
    t = np.asarray(targets)
    w = np.asarray(sample_weight, dtype=np.float32)
    pi = np.asarray(pair_i).astype(np.int64, copy=False)
    pj = np.asarray(pair_j).astype(np.int64, copy=False)

    li = x[pi]  # (NPAIRS, 5)
    lj = x[pj]
    dy = (t[pi] - t[pj]).astype(np.float32)
    wi = w[pi]
    wj = w[pj]

    PL = P * L
    bf = ml_dtypes.bfloat16
    maps = []
    for k in range(NCORES):
        sl = slice(k * PC, (k + 1) * PC)
        A = np.zeros((P, NPLANE, L), dtype=bf)

        def put(plane, v):
            vv = np.zeros(PL, dtype=np.float32)
            vv[:PC] = v
            A[:, plane, :] = vv.reshape(P, L).astype(bf)

        lis = li[sl]
        ljs = lj[sl]
        for ccls in range(5):
            put(ccls, lis[:, ccls])
            put(5 + ccls, ljs[:, ccls])
        put(10, dy[sl])
        put(11, wi[sl])
        put(12, wj[sl])
        maps.append({"x": A})
    return maps


def _run(in_maps, trace=False, **kw):
    nc = _get_nc()
    return run_bass_kernel_spmd(nc, in_maps, list(range(NCORES)), trace=trace, **kw)


def kernel(inputs, targets, cluster_ids, sample_weight, pair_i, pair_j):
    in_maps = _prepare(inputs, targets, cluster_ids, sample_weight, pair_i, pair_j)
    res = _run(in_maps)
    tl = 0.0
    tw = 0.0
    for k in range(NCORES):
        o = res.results[k]["out"]
        tl += float(o[:, 0].sum(dtype=np.float64))
        tw += float(o[:, 1].sum(dtype=np.float64))
    # the 0.5 pair-weight factor cancels in the ratio; fold it into eps
    return np.float32(tl / (tw + 2 * EPS))


# revision 12
# speedup vs baseline: 1.3836x; 1.1898x over previous
"""ClusterInversionLoss Trainium2 kernel.

Strategy (data-parallel over the flat pair list, per sharding hint):
  - Host: co-locate each pair's data by gathering rows at pair_i/pair_j
    (logits per class, target delta, sample weights), shard the 2M pairs
    evenly across 8 cores, lay out per-core planes as (128, 13, L) bf16.
  - Device (per core, pure streaming, no random access):
      s = softmax-expected-score of the 5 logits for both pair sides
      (exp on ACT; class-weighted sums via scalar_tensor_tensor FMAs;
      division via exp(-ln Z) to stay inside one ACT table set),
      softplus ranking loss via exp/ln with the free affine bias,
      fused multiply+reduce into per-partition accumulators.
  - Host: sum the 8x128 partial (loss, weight) pairs, return the ratio.

Computes exactly the reference quantity; only rows referenced by pairs
contribute to the loss, so unpaired rows need not be touched.
"""

import os

import numpy as np

import concourse.bacc as bacc
import concourse.mybir as mybir
from concourse.bass_utils import run_bass_kernel_spmd
from concourse.tile import TileContext

NCORES = 8
NPAIRS = 2_000_000
PC = NPAIRS // NCORES  # 250_000 pairs per core
P = 128
NCHUNK = 2
LC = 978
L = NCHUNK * LC  # 1956 columns per partition; P*L = 250_368 >= PC
NPLANE = 11  # li1'..li4', lj1'..lj4' (l0-shifted logits), dy, wi, wj

EPS = 1e-8

f32 = mybir.dt.float32
bf16 = mybir.dt.bfloat16
AF = mybir.ActivationFunctionType
ALU = mybir.AluOpType


def _pin_act_tables(arch):
    """Make every ACT function we use first-match to one table set that
    contains both exp and ln, so the kernel needs a single
    ACT_TABLE_LOAD instead of thrashing between the exp-only and
    ln-only sets (1.3us per reload).  Only membership of the cached
    selection dict is edited; set indices (act_func_set_id) and the
    real on-device tables are untouched, so lowering stays correct.
    """
    from concourse.hw_specs import get_activation_tables

    tabs = get_activation_tables(arch)
    ours = {AF.Exp, AF.Ln, AF.Sign, AF.Abs, AF.Square}
    combined = None
    for name, fns in tabs.items():
        if ours <= fns:
            combined = name
            break
    if combined is None:
        return
    for name, fns in tabs.items():
        if name != combined:
            fns -= ours


def _build():
    nc = bacc.Bacc("TRN2", target_bir_lowering=False)
    _pin_act_tables(nc.m.arch)
    X = nc.dram_tensor("x", [P, NPLANE, L], bf16, kind="ExternalInput")
    OUT = nc.dram_tensor("out", [P, 2], f32, kind="ExternalOutput")

    with TileContext(nc) as tc:
        with (
            tc.tile_pool(name="io", bufs=2) as io,
            tc.tile_pool(name="ew", bufs=2) as ew,
            tc.tile_pool(name="sc", bufs=2) as sc,
            tc.tile_pool(name="s1", bufs=1) as s1p,
            tc.tile_pool(name="acc", bufs=1) as accp,
        ):
            accL = [accp.tile([P, 1], f32, tag=f"accL{c}", name=f"accL{c}")
                    for c in range(NCHUNK)]
            accW = [accp.tile([P, 1], f32, tag=f"accW{c}", name=f"accW{c}")
                    for c in range(NCHUNK)]

            for c in range(NCHUNK):
                cs = slice(c * LC, (c + 1) * LC)

                LI = io.tile([P, 4, LC], bf16, tag="LI")
                nc.sync.dma_start(out=LI[:], in_=X[:, 0:4, cs])
                LJ = io.tile([P, 4, LC], bf16, tag="LJ")
                nc.sync.dma_start(out=LJ[:], in_=X[:, 4:8, cs])
                DY = io.tile([P, LC], bf16, tag="DY")
                nc.sync.dma_start(out=DY[:], in_=X[:, 8, cs])
                WI = io.tile([P, LC], bf16, tag="WI")
                nc.sync.dma_start(out=WI[:], in_=X[:, 9, cs])
                WJ = io.tile([P, LC], bf16, tag="WJ")
                nc.sync.dma_start(out=WJ[:], in_=X[:, 10, cs])

                # e_c = exp(l_c - l0) for c=1..4, one big ACT op per side
                EI = ew.tile([P, 4, LC], bf16, tag="EI")
                nc.scalar.activation(EI[:], LI[:], AF.Exp)
                EJ = ew.tile([P, 4, LC], bf16, tag="EJ")
                nc.scalar.activation(EJ[:], LJ[:], AF.Exp)

                # suffix-sum chains give Z and W in 6 adds + 1 tensor_scalar:
                #   A=e3+e4; B=e2+A; T=e1+B (=T1); Z=1+T1
                #   U=T1+B; V=A+e4; W=U+V = e1+2e2+3e3+4e4
                ZIJ = sc.tile([P, 2, LC], bf16, tag="ZIJ")
                WT = {}
                for side, (E, an, bn, tn) in enumerate(
                        ((EI, "Ai", "Bi", "Ti"), (EJ, "Aj", "Bj", "Tj"))):
                    A = sc.tile([P, LC], bf16, tag=an, name=an)
                    B = sc.tile([P, LC], bf16, tag=bn, name=bn)
                    T = sc.tile([P, LC], bf16, tag=tn, name=tn)
                    nc.vector.tensor_add(out=A[:], in0=E[:, 2, :], in1=E[:, 3, :])
                    nc.vector.tensor_add(out=B[:], in0=E[:, 1, :], in1=A[:])
                    nc.vector.tensor_add(out=T[:], in0=E[:, 0, :], in1=B[:])
                    nc.vector.tensor_scalar_add(out=ZIJ[:, side, :], in0=T[:],
                                                scalar1=1.0)
                    nc.vector.tensor_add(out=B[:], in0=T[:], in1=B[:])   # U
                    nc.vector.tensor_add(out=A[:], in0=A[:], in1=E[:, 3, :])  # V
                    nc.vector.tensor_add(out=T[:], in0=B[:], in1=A[:])   # W
                    WT[side] = T

                # 1/Z = exp(-ln Z) on ACT, both sides in one op pair
                ZT = s1p.tile([P, 2, LC], f32, tag="ZT")
                nc.scalar.activation(ZT[:], ZIJ[:], AF.Ln)
                RZ = s1p.tile([P, 2, LC], f32, tag="RZ")
                nc.scalar.activation(RZ[:], ZT[:], AF.Exp, scale=-1.0)
                SG = s1p.tile([P, LC], f32, tag="SG")
                nc.scalar.activation(SG[:], DY[:], AF.Sign)
                DIST = s1p.tile([P, LC], f32, tag="DIST")
                nc.scalar.activation(DIST[:], DY[:], AF.Abs)

                # s_i, s_j, ds, delta, wsum, coeff on Pool engine
                Si = s1p.tile([P, LC], f32, tag="Si")
                Sj = s1p.tile([P, LC], f32, tag="Sj")
                nc.gpsimd.tensor_mul(out=Si[:], in0=WT[0][:], in1=RZ[:, 0, :])
                nc.gpsimd.tensor_mul(out=Sj[:], in0=WT[1][:], in1=RZ[:, 1, :])
                nc.gpsimd.tensor_sub(out=Si[:], in0=Si[:], in1=Sj[:])
                S1 = s1p.tile([P, LC], f32, tag="S1")
                nc.gpsimd.tensor_mul(out=S1[:], in0=SG[:], in1=Si[:])
                WS = s1p.tile([P, LC], bf16, tag="WS")
                nc.gpsimd.tensor_add(out=WS[:], in0=WI[:], in1=WJ[:])
                S2C = s1p.tile([P, LC], f32, tag="S2C")
                nc.gpsimd.tensor_mul(out=S2C[:], in0=DIST[:], in1=WS[:])

                # softplus(-delta) = ln(1 + exp(-delta)) on ACT
                S2 = s1p.tile([P, LC], f32, tag="S2")
                nc.scalar.activation(S2[:], S1[:], AF.Exp, scale=-1.0)
                nc.scalar.activation(S1[:], S2[:], AF.Ln, bias=1.0)
                # active = min(dist, 1)
                S3 = s1p.tile([P, LC], f32, tag="S3")
                nc.vector.tensor_scalar(
                    out=S3[:], in0=DIST[:], scalar1=1.0, scalar2=None, op0=ALU.min)

                # fused multiply + per-partition reduce (0.5 folded on host)
                S5 = s1p.tile([P, LC], f32, tag="S5")
                nc.vector.scalar_tensor_tensor(
                    out=S5[:], in0=S1[:], scalar=1.0, in1=S2C[:],
                    op0=ALU.mult, op1=ALU.mult, accum_out=accL[c][:])
                nc.vector.scalar_tensor_tensor(
                    out=S1[:], in0=S3[:], scalar=1.0, in1=WS[:],
                    op0=ALU.mult, op1=ALU.mult, accum_out=accW[c][:])
                if c > 0:
                    nc.vector.tensor_add(out=accL[c][:], in0=accL[c][:],
                                         in1=accL[c - 1][:])
                    nc.vector.tensor_add(out=accW[c][:], in0=accW[c][:],
                                         in1=accW[c - 1][:])

            nc.sync.dma_start(out=OUT[:, 0:1], in_=accL[NCHUNK - 1][:])
            nc.sync.dma_start(out=OUT[:, 1:2], in_=accW[NCHUNK - 1][:])

    nc.compile()
    return nc


_NC_CACHE = {}


def _get_nc():
    if "nc" not in _NC_CACHE:
        _NC_CACHE["nc"] = _build()
    return _NC_CACHE["nc"]


def _prepare(inputs, targets, cluster_ids, sample_weight, pair_i, pair_j):
    import ml_dtypes

    x = np.ascontiguousarray(np.asarray(inputs), dtype=np.float32)
    t = np.asarray(targets)
    w = np.asarray(sample_weight, dtype=np.float32)
    pi = np.asarray(pair_i).astype(np.int64, copy=False)
    pj = np.asarray(pair_j).astype(np.int64, copy=False)

    li = x[pi]  # (NPAIRS, 5)
    lj = x[pj]
    lis = li[:, 1:5] - li[:, 0:1]  # l0-shift: softmax is shift-invariant
    ljs = lj[:, 1:5] - lj[:, 0:1]
    dy = (t[pi] - t[pj]).astype(np.float32)
    wi = w[pi]
    wj = w[pj]

    PL = P * L
    bf = ml_dtypes.bfloat16
    maps = []
    for k in range(NCORES):
        sl = slice(k * PC, (k + 1) * PC)
        A = np.zeros((P, NPLANE, L), dtype=bf)

        def put(plane, v):
            vv = np.zeros(PL, dtype=np.float32)
            vv[:PC] = v
            A[:, plane, :] = vv.reshape(P, L).astype(bf)

        for ccls in range(4):
            put(ccls, lis[sl][:, ccls])
            put(4 + ccls, ljs[sl][:, ccls])
        put(8, dy[sl])
        put(9, wi[sl])
        put(10, wj[sl])
        maps.append({"x": A})
    return maps


def _run(in_maps, trace=False, **kw):
    nc = _get_nc()
    return run_bass_kernel_spmd(nc, in_maps, list(range(NCORES)), trace=trace, **kw)


def kernel(inputs, targets, cluster_ids, sample_weight, pair_i, pair_j):
    in_maps = _prepare(inputs, targets, cluster_ids, sample_weight, pair_i, pair_j)
    res = _run(in_maps)
    tl = 0.0
    tw = 0.0
    for k in range(NCORES):
        o = res.results[k]["out"]
        tl += float(o[:, 0].sum(dtype=np.float64))
        tw += float(o[:, 1].sum(dtype=np.float64))
    # the 0.5 pair-weight factor cancels in the ratio; fold it into eps
    return np.float32(tl / (tw + 2 * EPS))
